# revision 24
# baseline (speedup 1.0000x reference)
"""GNN NodeBlock kernel for 8 TRN2 NeuronCores.

Math: out = (segment_mean(edge_attrs @ Wp + bp, dst)) @ Wu + bu.
Projection is linear, so it commutes with the segment sum: the two MLPs fuse
into one 64x64 weight Wf = Wp @ Wu (host-computed) applied to the per-node
aggregate. The mean's 1/count is a per-node scale that commutes all the way
back to the edge rows, so the host pre-scales each edge row by
1/count(dst(e)) and the device's plain segment-SUM yields the mean directly.
Biases reduce to a constant row added on the host (zero here).

Edge payloads ship as fp8 (e4m3) with host-side error-diffusion: along each
node's edge chain the quantization residual is carried into the next edge, so
the per-node fp32 sum sees only ONE rounding error instead of sqrt(count)
of them (measured 7.7e-3 end-to-end rel err vs 2.65e-2 for naive fp8).
This halves the dominant HBM traffic; the DMA pool is the roofline.

Sharding: node-parallel, equal node ranges per core. Each core's nodes map to
(tile, column) slots, 128 nodes per tile, C=16 base edge slots per node laid
out consecutively, so a tile is 16 chunks of 128 slots whose slot->column map
is the SAME aligned pattern (col = slot//16) for every chunk: one tiny [128,8]
0/1 constant is the moving operand of every base matmul (x chunk stationary,
fp8 x bf16 mixed dtypes -- allowed, only fp32 must match). Chunk j writes acc
columns 8j..8j+8 exactly once (start=True), so no zeroing and no split
matmuls. Edges beyond C slots ("spill", ~10%) go through data-dependent
onehot chunks (DVE is_equal vs an iota row, bf16 to keep DVE's 2x mode)
accumulated with start=False. The host concentrates spill-heavy nodes into
tiles NOSPILL_HEAD.. so warmup tiles need no spill; per-tile spill counts are
baked into the program (max across cores, SPMD).

PSUM batching: acc tiles for GA=4 consecutive node-tiles share one PSUM bank
[64, 512] and drain with ONE DVE copy (f32->bf16); MLP outputs for GO=8 tiles
share another bank [128, 512] and drain with ONE Act copy to the bf16 output
buffer, whose DMA (1KB/partition descriptors) is deferred one group. MLPs for
acc group g are emitted one group late so the PE never stalls on the copy.

Engine queues: base X stream on SP (ramped group sizes so the first tiles
land early), outputs + wf/a16 on Act, spill X + iota/sldst on gpsimd/SWDGE,
onehots + acc copies on DVE, everything matmul on PE.
"""

import os
import sys

sys.path.insert(0, "/opt/trn_rl_repo")

import numpy as np

P = 128
D = 64
NCORES = 8
C = int(os.environ.get("K_C", "16"))  # base edge slots per node (divides 128)
CH = C  # base chunks per 128-node tile
W = P // C  # nodes (pattern columns) per chunk
SQB = 16  # spill chunks per spill DMA group
MB = 4  # tiles per input X DMA
GA = int(os.environ.get("K_GA", "4"))  # tiles per PSUM acc bank / DVE copy
GO = int(os.environ.get("K_GO", "8"))  # tiles per PSUM mlp bank / out batch
BUF_XIN = int(os.environ.get("K_XIN", "3"))
BUF_ACC = int(os.environ.get("K_ACC", "3"))
BUF_SBF = int(os.environ.get("K_SBF", "3"))
BUF_MLP = int(os.environ.get("K_MLP", "2"))
TAIL2 = int(os.environ.get("K_TAIL2", "8"))  # last tiles loaded in twos
NOSPILL_HEAD = int(os.environ.get("K_HEAD", "6"))  # leading spill-free tiles


def _groups(nt, ramp, body):
    """Partition range(nt) into spans: `ramp` prefix then `body`-sized."""
    out = []
    t = 0
    ramp = list(ramp)
    while t < nt:
        span = ramp.pop(0) if ramp else body
        span = min(span, nt - t)
        out.append((t, span))
        t += span
    return out


def _build_program(nt, s_list):
    import concourse.bacc as bacc
    from concourse import mybir
    from concourse.tile import TileContext

    BF = mybir.dt.bfloat16
    F8 = mybir.dt.float8e4
    F32 = mybir.dt.float32
    nchunk = nt * CH
    qstart = np.concatenate([[0], np.cumsum(s_list)]).astype(int)
    nq = int(qstart[-1])

    nc = bacc.Bacc(None, target_bir_lowering=False)
    x_d = nc.declare_dram_parameter("x", [P, nchunk * D], F8, isOutput=False)
    sx_d = nc.declare_dram_parameter("sx", [P, max(nq, 1) * D], F8, isOutput=False)
    # one constant blob: [a16 | wf(rows 0..63) | iota], all bf16
    ncst = W + D + P
    cst_d = nc.declare_dram_parameter("cst", [P, ncst], BF, isOutput=False)
    sldst_d = nc.declare_dram_parameter("sldst", [P, max(nq, 1)], F32, isOutput=False)
    out_d = nc.declare_dram_parameter("out", [P, nt * D], BF, isOutput=True)

    # spill DMA groups of SQB chunks; group g covers q in [g*SQB, (g+1)*SQB)
    nsg = (nq + SQB - 1) // SQB
    first_tile = {}
    for t in range(nt):
        for q in range(qstart[t], qstart[t + 1]):
            g = q // SQB
            if g not in first_tile:
                first_tile[g] = t
    sched = {}
    for g in range(nsg):
        et = max(0, first_tile.get(g, 0) - 4)
        sched.setdefault(et, []).append(g)

    def _tailed(ramp, body, tail):
        """Spans over range(nt): ramp, then body-sized, then `tail` at the end."""
        tl = [s for s in tail if s]
        tsum = sum(tl)
        head = _groups(max(nt - tsum, 0), ramp, body)
        t0 = max(nt - tsum, 0)
        out = list(head)
        for s in tl:
            if t0 >= nt:
                break
            s = min(s, nt - t0)
            out.append((t0, s))
            t0 += s
        return out

    xgroups = _tailed([1, 1, 2], MB, [2, 2, 1, 1])
    agroups = _tailed([2, 2], GA, [2, 1, 1])
    ogroups = _tailed([4], GO, [4, 2, 1])
    xg_of = {}
    for g0, span in xgroups:
        for tt in range(g0, g0 + span):
            xg_of[tt] = (g0, span)
    ag_of = {}
    for g0, span in agroups:
        for tt in range(g0, g0 + span):
            ag_of[tt] = (g0, span)
    og_of = {}
    for g0, span in ogroups:
        for tt in range(g0, g0 + span):
            og_of[tt] = (g0, span)

    with TileContext(nc) as tc:
        with (
            tc.tile_pool(name="const", bufs=1) as cp,
            tc.tile_pool(name="xin", bufs=BUF_XIN) as xp,
            tc.tile_pool(name="sxin", bufs=2) as sxp,
            tc.tile_pool(name="oh", bufs=1) as ohp,
            tc.tile_pool(name="sbf", bufs=BUF_SBF) as fp,
            tc.tile_pool(name="res", bufs=2) as rp,
            tc.tile_pool(name="psacc", bufs=BUF_ACC, space="PSUM") as psa,
            tc.tile_pool(name="psmlp", bufs=BUF_MLP, space="PSUM") as psb,
        ):
            cst_sb = cp.tile([P, ncst], BF)
            nc.scalar.dma_start(out=cst_sb[:], in_=cst_d[:])
            sldst_sb = cp.tile([P, max(nq, 1)], F32)
            nc.scalar.dma_start(out=sldst_sb[:], in_=sldst_d[:])

            def a16_ap():
                return cst_sb[:, 0:W]

            def wf_ap():
                return cst_sb[0:D, W : W + D]

            def iota_ap():
                return cst_sb[:, W + D : W + D + P]

            def sldst_ap(q):
                return sldst_sb[:, q : q + 1]

            xins = {}
            sxins = {}
            ohs = {}
            accs = {}
            sbfs = {}
            mlps = {}
            outbs = {}
            ocount = {}
            pending_out = []
            mlp_backlog = []

            def emit_spill_groups(t):
                for g in sched.get(t, []):
                    q0 = g * SQB
                    span = min(SQB, nq - q0)
                    sxin = sxp.tile([P, SQB * D], F8, tag="sxin", name=f"sxin{g}")
                    nc.gpsimd.dma_start(
                        out=sxin[:, : span * D],
                        in_=sx_d[:, q0 * D : (q0 + span) * D],
                    )
                    sxins[g] = sxin

            def emit_onehots():
                # Onehots are pure functions of constants -- generate ALL of
                # them upfront so no DVE copy ever blocks one in queue order.
                for q in range(nq):
                    oh = ohp.tile([P, P], BF, name=f"oh{q}")
                    nc.vector.tensor_scalar(
                        out=oh[:],
                        in0=iota_ap(),
                        scalar1=sldst_ap(q),
                        scalar2=None,
                        op0=mybir.AluOpType.is_equal,
                    )
                    ohs[q] = oh

            def emit_mlp(t):
                """MLP for tile t (reads sbf of t's acc group); on out-group
                completion: Act copy to bf16 and deferred out DMA flush."""
                og0, ospan = og_of[t]
                if og0 not in mlps:
                    mlps[og0] = psb.tile([P, GO * D], F32, tag="mlp", name=f"mlp{og0}")
                    ocount[og0] = 0
                a0, _ = ag_of[t]
                nc.tensor.matmul(
                    mlps[og0][:, (t - og0) * D : (t - og0 + 1) * D],
                    lhsT=sbfs[a0][:, (t - a0) * P : (t - a0 + 1) * P],
                    rhs=wf_ap(),
                    start=True,
                    stop=True,
                )
                ocount[og0] += 1
                if ocount[og0] == ospan:
                    outb = rp.tile([P, GO * D], BF, tag="outb", name=f"outb{og0}")
                    nc.scalar.activation(
                        out=outb[:, : ospan * D],
                        in_=mlps.pop(og0)[:, : ospan * D],
                        func=mybir.ActivationFunctionType.Copy,
                    )
                    outbs[og0] = outb
                    pending_out.append((og0, ospan))
                    while len(pending_out) > 1:
                        f0, fspan = pending_out.pop(0)
                        nc.scalar.dma_start(
                            out=out_d[:, f0 * D : (f0 + fspan) * D],
                            in_=outbs.pop(f0)[:, : fspan * D],
                        )

            emit_spill_groups(0)
            emit_onehots()
            for t in range(nt):
                xg0, xspan = xg_of[t]
                if xg0 == t:
                    xin = xp.tile([P, MB * CH * D], F8, tag="xin", name=f"xin{t}")
                    nc.sync.dma_start(
                        out=xin[:, : xspan * CH * D],
                        in_=x_d[:, t * CH * D : (t + xspan) * CH * D],
                    )
                    xins[t] = xin
                if t > 0:
                    emit_spill_groups(t)
                xin = xins[xg0]
                xoff = (t - xg0) * CH * D

                a0, aspan = ag_of[t]
                if a0 == t:
                    accs[a0] = psa.tile([D, GA * P], F32, tag="acc", name=f"acc{a0}")
                acc = accs[a0]
                coff = (t - a0) * P
                ns = s_list[t]
                # start=True pending-marks the WHOLE 2KB PSUM bank (zero
                # region), so exactly ONE start per bank: the group's first
                # matmul. Later matmuls overwrite pending bytes / accumulate
                # cleared ones. One stop on the group's last matmul.
                last_tile = t == a0 + aspan - 1
                for j in range(CH):
                    nc.tensor.matmul(
                        acc[:, coff + j * W : coff + (j + 1) * W],
                        lhsT=xin[:, xoff + j * D : xoff + (j + 1) * D],
                        rhs=a16_ap(),
                        start=(t == a0 and j == 0),
                        stop=(last_tile and ns == 0 and j == CH - 1),
                        skip_group_check=True,
                    )
                for s in range(ns):
                    q = int(qstart[t]) + s
                    sg = q // SQB
                    nc.tensor.matmul(
                        acc[:, coff : coff + P],
                        lhsT=sxins[sg][:, (q - sg * SQB) * D : (q - sg * SQB + 1) * D],
                        rhs=ohs[q][:],
                        start=False,
                        stop=(last_tile and s == ns - 1),
                        skip_group_check=True,
                    )

                if t == a0 + aspan - 1:
                    s_bf = fp.tile([D, GA * P], BF, tag="sbf", name=f"sbf{a0}")
                    nc.vector.tensor_copy(
                        s_bf[:, : aspan * P], accs.pop(a0)[:, : aspan * P]
                    )
                    sbfs[a0] = s_bf
                    # MLPs for the PREVIOUS acc group (deferred one group)
                    if mlp_backlog:
                        pa0, paspan = mlp_backlog.pop(0)
                        for tt in range(pa0, pa0 + paspan):
                            emit_mlp(tt)
                    mlp_backlog.append((a0, aspan))

            for pa0, paspan in mlp_backlog:
                for tt in range(pa0, pa0 + paspan):
                    emit_mlp(tt)
            for f0, fspan in pending_out:
                nc.scalar.dma_start(
                    out=out_d[:, f0 * D : (f0 + fspan) * D],
                    in_=outbs.pop(f0)[:, : fspan * D],
                )

    return nc


def _prepare(inputs):
    """Host-side shard/layout prep.

    Returns (in_maps, meta); meta = (node_of [NCORES, nt, P], nt, s_list,
    bias, bias0, counts, N).
    """
    from concourse import mybir

    bf16 = mybir.dt.np(mybir.dt.bfloat16)
    f8 = mybir.dt.np(mybir.dt.float8e4)

    edge_attrs = np.asarray(inputs["edge_attrs"], dtype=np.float32)
    wp = np.asarray(inputs["proj_W"], dtype=np.float32)
    bp = np.asarray(inputs.get("proj_b", np.zeros(D)), dtype=np.float32)
    wu = np.asarray(inputs["upd_W"], dtype=np.float32)
    bu = np.asarray(inputs.get("upd_b", np.zeros(D)), dtype=np.float32)
    dst = np.asarray(inputs["dst"]).astype(np.int64).ravel()
    N = int(np.asarray(inputs["n_nodes"]))
    E = dst.shape[0]

    wf = np.ascontiguousarray(wp @ wu)
    bias = bp @ wu + bu  # added to nodes with >=1 edge
    bias0 = bu.copy()  # value for nodes with no edges

    perm = np.argsort(dst, kind="stable")
    sdst = dst[perm]
    sx = edge_attrs[perm]

    counts = np.bincount(sdst, minlength=N).astype(np.int64)
    cum = np.concatenate([[0], np.cumsum(counts)])
    rank = np.arange(E, dtype=np.int64) - cum[sdst]
    recip_g = (1.0 / np.maximum(counts, 1)).astype(np.float32)

    # Pre-scale each edge row by 1/count so the device sum IS the mean, then
    # fp8-quantize with error diffusion along each node's edge chain (the
    # residual rides into the next edge, so the node sum sees one rounding).
    sxs = sx * recip_g[sdst][:, None]
    q8 = np.empty((E, D), dtype=f8)
    err = np.zeros((N, D), np.float32)
    idx0 = cum[:-1]
    for r in range(int(counts.max(initial=0))):
        m = counts > r
        idx = idx0[m] + r
        v = sxs[idx] + err[m]
        qv = v.astype(f8)
        q8[idx] = qv
        err[m] = v - qv.astype(np.float32)

    npc = (N + NCORES - 1) // NCORES
    nt = (npc + P - 1) // P

    # --- per-core planning: concentrate spill-heavy nodes into early tiles ---
    core_plans = []
    s_need = np.zeros((NCORES, nt), dtype=np.int64)
    for k in range(NCORES):
        g0, g1 = k * npc, min((k + 1) * npc, N)
        nloc = g1 - g0
        cnt = counts[g0:g1]
        spill_n = np.maximum(cnt - C, 0)
        cap = max(2 * P, int(spill_n.max(initial=0)))
        order = np.argsort(-spill_n, kind="stable")
        tile_of = np.empty(nloc, dtype=np.int64)
        col_of = np.empty(nloc, dtype=np.int64)
        used = np.zeros(nt, dtype=np.int64)
        tspill = np.zeros(nt, dtype=np.int64)
        head = min(NOSPILL_HEAD, max(nt - 2, 0))
        cand = list(range(head, nt))
        caps = np.zeros(nt, dtype=np.int64)
        caps[head:] = cap
        oi = 0
        nnz = int((spill_n > 0).sum())
        for n in order[:nnz]:
            sp = spill_n[n]
            while True:
                if oi >= len(cand):
                    caps[head:] += P
                    oi = 0
                ti = cand[oi]
                if used[ti] < P and tspill[ti] + sp <= caps[ti]:
                    break
                oi += 1
            tile_of[n] = ti
            col_of[n] = used[ti]
            used[ti] += 1
            tspill[ti] += sp
        free = P - used
        ztiles = np.repeat(np.arange(nt), free)
        zcols = np.concatenate([np.arange(used[t], P) for t in range(nt)]) if nt else np.array([], dtype=np.int64)
        zn = order[nnz:]
        tile_of[zn] = ztiles[: zn.shape[0]]
        col_of[zn] = zcols[: zn.shape[0]]
        s_need[k] = (tspill + P - 1) // P
        core_plans.append((g0, g1, tile_of, col_of))
    s_list = s_need.max(axis=0).astype(int).tolist()
    qstart = np.concatenate([[0], np.cumsum(s_list)]).astype(np.int64)
    nq = int(qstart[-1])

    nchunk = nt * CH
    a16 = np.zeros((P, W), dtype=np.float32)
    a16[np.arange(P), np.arange(P) // C] = 1.0
    iota = np.broadcast_to(np.arange(P, dtype=np.float32), (P, P))
    wf_pad = np.zeros((P, D), dtype=np.float32)
    wf_pad[:D] = wf

    in_maps = []
    node_of_all = np.full((NCORES, nt, P), -1, dtype=np.int64)
    for k in range(NCORES):
        g0, g1, tile_of, col_of = core_plans[k]
        node_of_all[k, tile_of, col_of] = np.arange(g0, g1)

        e0, e1 = int(cum[g0]), int(cum[g1])
        ed = sdst[e0:e1] - g0
        er = rank[e0:e1]
        et = tile_of[ed]
        ec = col_of[ed]

        x_base = np.zeros((nchunk * P, D), dtype=f8)
        base_m = er < C
        slot = et * (CH * P) + ec * C + er
        x_base[slot[base_m]] = q8[e0:e1][base_m]

        x_spill = np.zeros((max(nq, 1) * P, D), dtype=f8)
        sldst = np.full((max(nq, 1) * P,), 1000.0, dtype=np.float32)
        sp_m = ~base_m
        sp_t = et[sp_m]
        order2 = np.argsort(sp_t, kind="stable")
        sp_rank = np.arange(sp_t.shape[0], dtype=np.int64)
        tile_start = np.searchsorted(sp_t[order2], np.arange(nt))
        sp_rank_sorted = sp_rank - tile_start[sp_t[order2]]
        sslot = np.empty_like(sp_rank)
        sslot[order2] = (qstart[sp_t[order2]] + sp_rank_sorted // P) * P + sp_rank_sorted % P
        assert sp_rank_sorted.max(initial=0) < np.asarray(s_list)[sp_t[order2]].max(initial=1) * P
        x_spill[sslot] = q8[e0:e1][sp_m]
        sldst[sslot] = ec[sp_m]

        x_dev = np.ascontiguousarray(
            x_base.reshape(nchunk, P, D).transpose(1, 0, 2).reshape(P, nchunk * D)
        )
        sx_dev = np.ascontiguousarray(
            x_spill.reshape(max(nq, 1), P, D).transpose(1, 0, 2).reshape(P, max(nq, 1) * D)
        )
        sldst_dev = np.ascontiguousarray(sldst.reshape(max(nq, 1), P).T)  # [P, nq]
        cst = np.concatenate([a16, wf_pad, iota], axis=1).astype(bf16)

        in_maps.append({"x": x_dev, "sx": sx_dev, "cst": cst, "sldst": sldst_dev})

    meta = (node_of_all, nt, s_list, bias, bias0, counts, N)
    return in_maps, meta


def _gather(results, meta):
    node_of_all, nt, s_list, bias, bias0, counts, N = meta
    out_full = np.zeros((N, D), dtype=np.float32)
    for k in range(NCORES):
        o = np.asarray(results[k]["out"], dtype=np.float32)  # [P, nt*D]
        o = o.reshape(P, nt, D).transpose(1, 0, 2)  # [nt, P, D]
        nid = node_of_all[k]
        m = nid >= 0
        out_full[nid[m]] = o[m]
    has_edge = counts > 0
    out_full[has_edge] += bias
    out_full[~has_edge] = bias0
    return out_full


def kernel(**inputs) -> np.ndarray:
    from concourse.bass_utils import run_bass_kernel_spmd

    in_maps, meta = _prepare(inputs)
    nt, s_list = meta[1], meta[2]
    nc = _build_program(nt, s_list)
    nc.finalize()
    res = run_bass_kernel_spmd(nc, in_maps, core_ids=list(range(NCORES)))
    return _gather(res.results, meta)


# revision 27
# speedup vs baseline: 1.1123x; 1.1123x over previous
"""GNN NodeBlock kernel for 8 TRN2 NeuronCores.

Math: out = (segment_mean(edge_attrs @ Wp + bp, dst)) @ Wu + bu.
Projection is linear, so it commutes with the segment sum: the two MLPs fuse
into one 64x64 weight Wf = Wp @ Wu (host-computed) applied to the per-node
aggregate. The mean's 1/count is a per-node scale that commutes all the way
back to the edge rows, so the host pre-scales each edge row by
1/count(dst(e)) and the device's plain segment-SUM yields the mean directly.
Biases reduce to a constant row added on the host (zero here).

Edge payloads ship as fp8 (e4m3) with host-side error-diffusion: along each
node's edge chain the quantization residual is carried into the next edge, so
the per-node fp32 sum sees only ONE rounding error instead of sqrt(count)
of them (measured 7.7e-3 end-to-end rel err vs 2.65e-2 for naive fp8).
This halves the dominant HBM traffic; the DMA pool is the roofline.

Sharding: node-parallel, equal node ranges per core. Each core's nodes map to
(tile, column) slots, 128 nodes per tile, C=16 base edge slots per node laid
out consecutively, so a tile is 16 chunks of 128 slots whose slot->column map
is the SAME aligned pattern (col = slot//16) for every chunk: one tiny [128,8]
0/1 constant is the moving operand of every base matmul (x chunk stationary,
fp8 x bf16 mixed dtypes -- allowed, only fp32 must match). Chunk j writes acc
columns 8j..8j+8 exactly once (start=True), so no zeroing and no split
matmuls. Edges beyond C slots ("spill", ~10%) go through data-dependent
onehot chunks (DVE is_equal vs an iota row, bf16 to keep DVE's 2x mode)
accumulated with start=False. The host concentrates spill-heavy nodes into
tiles NOSPILL_HEAD.. so warmup tiles need no spill; per-tile spill counts are
baked into the program (max across cores, SPMD).

PSUM batching: acc tiles for GA=4 consecutive node-tiles share one PSUM bank
[64, 512] and drain with ONE DVE copy (f32->bf16); MLP outputs for GO=8 tiles
share another bank [128, 512] and drain with ONE Act copy to the bf16 output
buffer, whose DMA (1KB/partition descriptors) is deferred one group. MLPs for
acc group g are emitted one group late so the PE never stalls on the copy.

Engine queues: base X stream on SP (ramped group sizes so the first tiles
land early), outputs + wf/a16 on Act, spill X + iota/sldst on gpsimd/SWDGE,
onehots + acc copies on DVE, everything matmul on PE.
"""

import os
import sys

sys.path.insert(0, "/opt/trn_rl_repo")

import numpy as np

P = 128
D = 64
NCORES = 8
C = int(os.environ.get("K_C", "16"))  # base edge slots per node (divides 128)
CH = C  # base chunks per 128-node tile
W = P // C  # nodes (pattern columns) per chunk
SQB = 16  # spill chunks per spill DMA group
MB = 4  # tiles per input X DMA
GA = int(os.environ.get("K_GA", "4"))  # tiles per PSUM acc bank / DVE copy
GO = int(os.environ.get("K_GO", "8"))  # tiles per PSUM mlp bank / out batch
BUF_XIN = int(os.environ.get("K_XIN", "3"))
BUF_ACC = int(os.environ.get("K_ACC", "3"))
BUF_SBF = int(os.environ.get("K_SBF", "3"))
BUF_MLP = int(os.environ.get("K_MLP", "2"))
TAIL2 = int(os.environ.get("K_TAIL2", "8"))  # last tiles loaded in twos
NOSPILL_HEAD = int(os.environ.get("K_HEAD", "6"))  # leading spill-free tiles
LEAD = int(os.environ.get("K_LEAD", "10"))  # onehot generation lead (tiles)


def _groups(nt, ramp, body):
    """Partition range(nt) into spans: `ramp` prefix then `body`-sized."""
    out = []
    t = 0
    ramp = list(ramp)
    while t < nt:
        span = ramp.pop(0) if ramp else body
        span = min(span, nt - t)
        out.append((t, span))
        t += span
    return out


def _build_program(nt, s_list):
    import concourse.bacc as bacc
    from concourse import mybir
    from concourse.tile import TileContext

    BF = mybir.dt.bfloat16
    F8 = mybir.dt.float8e4
    F32 = mybir.dt.float32
    nchunk = nt * CH
    qstart = np.concatenate([[0], np.cumsum(s_list)]).astype(int)
    nq = int(qstart[-1])

    nc = bacc.Bacc(None, target_bir_lowering=False)
    x_d = nc.declare_dram_parameter("x", [P, nchunk * D], F8, isOutput=False)
    sx_d = nc.declare_dram_parameter("sx", [P, max(nq, 1) * D], F8, isOutput=False)
    # one constant blob: [a16 | wf(rows 0..63) | iota], all bf16
    ncst = W + D + P
    cst_d = nc.declare_dram_parameter("cst", [P, ncst], BF, isOutput=False)
    sldst_d = nc.declare_dram_parameter("sldst", [P, max(nq, 1)], F32, isOutput=False)
    out_d = nc.declare_dram_parameter("out", [P, nt * D], BF, isOutput=True)

    # spill DMA groups of SQB chunks; group g covers q in [g*SQB, (g+1)*SQB)
    nsg = (nq + SQB - 1) // SQB
    first_tile = {}
    for t in range(nt):
        for q in range(qstart[t], qstart[t + 1]):
            g = q // SQB
            if g not in first_tile:
                first_tile[g] = t
    sched = {}
    for g in range(nsg):
        et = max(0, first_tile.get(g, 0) - 4)
        sched.setdefault(et, []).append(g)

    def _tailed(ramp, body, tail):
        """Spans over range(nt): ramp, then body-sized, then `tail` at the end."""
        tl = [s for s in tail if s]
        tsum = sum(tl)
        head = _groups(max(nt - tsum, 0), ramp, body)
        t0 = max(nt - tsum, 0)
        out = list(head)
        for s in tl:
            if t0 >= nt:
                break
            s = min(s, nt - t0)
            out.append((t0, s))
            t0 += s
        return out

    xgroups = _tailed([1, 1, 2], MB, [2, 2, 1, 1])
    agroups = _tailed([2, 2], GA, [2, 1, 1])
    ogroups = _tailed([4], GO, [4, 2, 1])
    xg_of = {}
    for g0, span in xgroups:
        for tt in range(g0, g0 + span):
            xg_of[tt] = (g0, span)
    ag_of = {}
    for g0, span in agroups:
        for tt in range(g0, g0 + span):
            ag_of[tt] = (g0, span)
    og_of = {}
    for g0, span in ogroups:
        for tt in range(g0, g0 + span):
            og_of[tt] = (g0, span)

    with TileContext(nc) as tc:
        with (
            tc.tile_pool(name="const", bufs=1) as cp,
            tc.tile_pool(name="xin", bufs=BUF_XIN) as xp,
            tc.tile_pool(name="sxin", bufs=2) as sxp,
            tc.tile_pool(name="oh", bufs=1) as ohp,
            tc.tile_pool(name="sbf", bufs=BUF_SBF) as fp,
            tc.tile_pool(name="res", bufs=2) as rp,
            tc.tile_pool(name="psacc", bufs=BUF_ACC, space="PSUM") as psa,
            tc.tile_pool(name="psmlp", bufs=BUF_MLP, space="PSUM") as psb,
        ):
            cst_sb = cp.tile([P, ncst], BF)
            nc.scalar.dma_start(out=cst_sb[:], in_=cst_d[:])
            sldst_sb = cp.tile([P, max(nq, 1)], F32)
            nc.scalar.dma_start(out=sldst_sb[:], in_=sldst_d[:])

            def a16_ap():
                return cst_sb[:, 0:W]

            def wf_ap():
                return cst_sb[0:D, W : W + D]

            def iota_ap():
                return cst_sb[:, W + D : W + D + P]

            def sldst_ap(q):
                return sldst_sb[:, q : q + 1]

            xins = {}
            sxins = {}
            ohs = {}
            accs = {}
            sbfs = {}
            mlps = {}
            outbs = {}
            ocount = {}
            pending_out = []
            mlp_backlog = []

            def emit_spill_groups(t):
                for g in sched.get(t, []):
                    q0 = g * SQB
                    span = min(SQB, nq - q0)
                    sxin = sxp.tile([P, SQB * D], F8, tag="sxin", name=f"sxin{g}")
                    nc.gpsimd.dma_start(
                        out=sxin[:, : span * D],
                        in_=sx_d[:, q0 * D : (q0 + span) * D],
                    )
                    sxins[g] = sxin

            def emit_onehots_until(t):
                # Onehots depend only on constants; keep them LEAD tiles
                # ahead of the PE so an acc copy queued on DVE never makes
                # the PE wait for a onehot.
                q_hi = int(qstart[min(t + 1, nt)])
                for q in range(len(ohs), q_hi):
                    oh = ohp.tile([P, P], BF, name=f"oh{q}")
                    nc.vector.tensor_scalar(
                        out=oh[:],
                        in0=iota_ap(),
                        scalar1=sldst_ap(q),
                        scalar2=None,
                        op0=mybir.AluOpType.is_equal,
                    )
                    ohs[q] = oh

            def emit_mlp(t):
                """MLP for tile t (reads sbf of t's acc group); on out-group
                completion: Act copy to bf16 and deferred out DMA flush."""
                og0, ospan = og_of[t]
                if og0 not in mlps:
                    mlps[og0] = psb.tile([P, GO * D], F32, tag="mlp", name=f"mlp{og0}")
                    ocount[og0] = 0
                a0, _ = ag_of[t]
                nc.tensor.matmul(
                    mlps[og0][:, (t - og0) * D : (t - og0 + 1) * D],
                    lhsT=sbfs[a0][:, (t - a0) * P : (t - a0 + 1) * P],
                    rhs=wf_ap(),
                    start=True,
                    stop=True,
                )
                ocount[og0] += 1
                if ocount[og0] == ospan:
                    outb = rp.tile([P, GO * D], BF, tag="outb", name=f"outb{og0}")
                    nc.scalar.activation(
                        out=outb[:, : ospan * D],
                        in_=mlps.pop(og0)[:, : ospan * D],
                        func=mybir.ActivationFunctionType.Copy,
                    )
                    outbs[og0] = outb
                    pending_out.append((og0, ospan))
                    while len(pending_out) > 1:
                        f0, fspan = pending_out.pop(0)
                        nc.scalar.dma_start(
                            out=out_d[:, f0 * D : (f0 + fspan) * D],
                            in_=outbs.pop(f0)[:, : fspan * D],
                        )

            emit_spill_groups(0)
            for t in range(nt):
                emit_onehots_until(t + LEAD)
                xg0, xspan = xg_of[t]
                if xg0 == t:
                    xin = xp.tile([P, MB * CH * D], F8, tag="xin", name=f"xin{t}")
                    nc.sync.dma_start(
                        out=xin[:, : xspan * CH * D],
                        in_=x_d[:, t * CH * D : (t + xspan) * CH * D],
                    )
                    xins[t] = xin
                if t > 0:
                    emit_spill_groups(t)
                xin = xins[xg0]
                xoff = (t - xg0) * CH * D

                a0, aspan = ag_of[t]
                if a0 == t:
                    accs[a0] = psa.tile([D, GA * P], F32, tag="acc", name=f"acc{a0}")
                acc = accs[a0]
                coff = (t - a0) * P
                ns = s_list[t]
                # start=True pending-marks the WHOLE 2KB PSUM bank (zero
                # region), so exactly ONE start per bank: the group's first
                # matmul. Later matmuls overwrite pending bytes / accumulate
                # cleared ones. One stop on the group's last matmul.
                last_tile = t == a0 + aspan - 1
                for j in range(CH):
                    nc.tensor.matmul(
                        acc[:, coff + j * W : coff + (j + 1) * W],
                        lhsT=xin[:, xoff + j * D : xoff + (j + 1) * D],
                        rhs=a16_ap(),
                        start=(t == a0 and j == 0),
                        stop=(last_tile and ns == 0 and j == CH - 1),
                        skip_group_check=True,
                    )
                for s in range(ns):
                    q = int(qstart[t]) + s
                    sg = q // SQB
                    nc.tensor.matmul(
                        acc[:, coff : coff + P],
                        lhsT=sxins[sg][:, (q - sg * SQB) * D : (q - sg * SQB + 1) * D],
                        rhs=ohs[q][:],
                        start=False,
                        stop=(last_tile and s == ns - 1),
                        skip_group_check=True,
                    )

                if t == a0 + aspan - 1:
                    s_bf = fp.tile([D, GA * P], BF, tag="sbf", name=f"sbf{a0}")
                    nc.vector.tensor_copy(
                        s_bf[:, : aspan * P], accs.pop(a0)[:, : aspan * P]
                    )
                    sbfs[a0] = s_bf
                    # MLPs for the PREVIOUS acc group (deferred one group)
                    if mlp_backlog:
                        pa0, paspan = mlp_backlog.pop(0)
                        for tt in range(pa0, pa0 + paspan):
                            emit_mlp(tt)
                    mlp_backlog.append((a0, aspan))

            for pa0, paspan in mlp_backlog:
                for tt in range(pa0, pa0 + paspan):
                    emit_mlp(tt)
            for f0, fspan in pending_out:
                nc.scalar.dma_start(
                    out=out_d[:, f0 * D : (f0 + fspan) * D],
                    in_=outbs.pop(f0)[:, : fspan * D],
                )

    return nc


def _prepare(inputs):
    """Host-side shard/layout prep.

    Returns (in_maps, meta); meta = (node_of [NCORES, nt, P], nt, s_list,
    bias, bias0, counts, N).
    """
    from concourse import mybir

    bf16 = mybir.dt.np(mybir.dt.bfloat16)
    f8 = mybir.dt.np(mybir.dt.float8e4)

    edge_attrs = np.asarray(inputs["edge_attrs"], dtype=np.float32)
    wp = np.asarray(inputs["proj_W"], dtype=np.float32)
    bp = np.asarray(inputs.get("proj_b", np.zeros(D)), dtype=np.float32)
    wu = np.asarray(inputs["upd_W"], dtype=np.float32)
    bu = np.asarray(inputs.get("upd_b", np.zeros(D)), dtype=np.float32)
    dst = np.asarray(inputs["dst"]).astype(np.int64).ravel()
    N = int(np.asarray(inputs["n_nodes"]))
    E = dst.shape[0]

    wf = np.ascontiguousarray(wp @ wu)
    bias = bp @ wu + bu  # added to nodes with >=1 edge
    bias0 = bu.copy()  # value for nodes with no edges

    perm = np.argsort(dst, kind="stable")
    sdst = dst[perm]
    sx = edge_attrs[perm]

    counts = np.bincount(sdst, minlength=N).astype(np.int64)
    cum = np.concatenate([[0], np.cumsum(counts)])
    rank = np.arange(E, dtype=np.int64) - cum[sdst]
    recip_g = (1.0 / np.maximum(counts, 1)).astype(np.float32)

    # Pre-scale each edge row by 1/count so the device sum IS the mean, then
    # fp8-quantize with error diffusion along each node's edge chain (the
    # residual rides into the next edge, so the node sum sees one rounding).
    sxs = sx * recip_g[sdst][:, None]
    q8 = np.empty((E, D), dtype=f8)
    err = np.zeros((N, D), np.float32)
    idx0 = cum[:-1]
    for r in range(int(counts.max(initial=0))):
        m = counts > r
        idx = idx0[m] + r
        v = sxs[idx] + err[m]
        qv = v.astype(f8)
        q8[idx] = qv
        err[m] = v - qv.astype(np.float32)

    npc = (N + NCORES - 1) // NCORES
    nt = (npc + P - 1) // P

    # --- per-core planning: concentrate spill-heavy nodes into early tiles ---
    core_plans = []
    s_need = np.zeros((NCORES, nt), dtype=np.int64)
    for k in range(NCORES):
        g0, g1 = k * npc, min((k + 1) * npc, N)
        nloc = g1 - g0
        cnt = counts[g0:g1]
        spill_n = np.maximum(cnt - C, 0)
        cap = max(2 * P, int(spill_n.max(initial=0)))
        order = np.argsort(-spill_n, kind="stable")
        tile_of = np.empty(nloc, dtype=np.int64)
        col_of = np.empty(nloc, dtype=np.int64)
        used = np.zeros(nt, dtype=np.int64)
        tspill = np.zeros(nt, dtype=np.int64)
        head = min(NOSPILL_HEAD, max(nt - 2, 0))
        cand = list(range(head, nt))
        caps = np.zeros(nt, dtype=np.int64)
        caps[head:] = cap
        oi = 0
        nnz = int((spill_n > 0).sum())
        for n in order[:nnz]:
            sp = spill_n[n]
            while True:
                if oi >= len(cand):
                    caps[head:] += P
                    oi = 0
                ti = cand[oi]
                if used[ti] < P and tspill[ti] + sp <= caps[ti]:
                    break
                oi += 1
            tile_of[n] = ti
            col_of[n] = used[ti]
            used[ti] += 1
            tspill[ti] += sp
        free = P - used
        ztiles = np.repeat(np.arange(nt), free)
        zcols = np.concatenate([np.arange(used[t], P) for t in range(nt)]) if nt else np.array([], dtype=np.int64)
        zn = order[nnz:]
        tile_of[zn] = ztiles[: zn.shape[0]]
        col_of[zn] = zcols[: zn.shape[0]]
        s_need[k] = (tspill + P - 1) // P
        core_plans.append((g0, g1, tile_of, col_of))
    s_list = s_need.max(axis=0).astype(int).tolist()
    qstart = np.concatenate([[0], np.cumsum(s_list)]).astype(np.int64)
    nq = int(qstart[-1])

    nchunk = nt * CH
    a16 = np.zeros((P, W), dtype=np.float32)
    a16[np.arange(P), np.arange(P) // C] = 1.0
    iota = np.broadcast_to(np.arange(P, dtype=np.float32), (P, P))
    wf_pad = np.zeros((P, D), dtype=np.float32)
    wf_pad[:D] = wf

    in_maps = []
    node_of_all = np.full((NCORES, nt, P), -1, dtype=np.int64)
    for k in range(NCORES):
        g0, g1, tile_of, col_of = core_plans[k]
        node_of_all[k, tile_of, col_of] = np.arange(g0, g1)

        e0, e1 = int(cum[g0]), int(cum[g1])
        ed = sdst[e0:e1] - g0
        er = rank[e0:e1]
        et = tile_of[ed]
        ec = col_of[ed]

        x_base = np.zeros((nchunk * P, D), dtype=f8)
        base_m = er < C
        slot = et * (CH * P) + ec * C + er
        x_base[slot[base_m]] = q8[e0:e1][base_m]

        x_spill = np.zeros((max(nq, 1) * P, D), dtype=f8)
        sldst = np.full((max(nq, 1) * P,), 1000.0, dtype=np.float32)
        sp_m = ~base_m
        sp_t = et[sp_m]
        order2 = np.argsort(sp_t, kind="stable")
        sp_rank = np.arange(sp_t.shape[0], dtype=np.int64)
        tile_start = np.searchsorted(sp_t[order2], np.arange(nt))
        sp_rank_sorted = sp_rank - tile_start[sp_t[order2]]
        sslot = np.empty_like(sp_rank)
        sslot[order2] = (qstart[sp_t[order2]] + sp_rank_sorted // P) * P + sp_rank_sorted % P
        assert sp_rank_sorted.max(initial=0) < np.asarray(s_list)[sp_t[order2]].max(initial=1) * P
        x_spill[sslot] = q8[e0:e1][sp_m]
        sldst[sslot] = ec[sp_m]

        x_dev = np.ascontiguousarray(
            x_base.reshape(nchunk, P, D).transpose(1, 0, 2).reshape(P, nchunk * D)
        )
        sx_dev = np.ascontiguousarray(
            x_spill.reshape(max(nq, 1), P, D).transpose(1, 0, 2).reshape(P, max(nq, 1) * D)
        )
        sldst_dev = np.ascontiguousarray(sldst.reshape(max(nq, 1), P).T)  # [P, nq]
        cst = np.concatenate([a16, wf_pad, iota], axis=1).astype(bf16)

        in_maps.append({"x": x_dev, "sx": sx_dev, "cst": cst, "sldst": sldst_dev})

    meta = (node_of_all, nt, s_list, bias, bias0, counts, N)
    return in_maps, meta


def _gather(results, meta):
    node_of_all, nt, s_list, bias, bias0, counts, N = meta
    out_full = np.zeros((N, D), dtype=np.float32)
    for k in range(NCORES):
        o = np.asarray(results[k]["out"], dtype=np.float32)  # [P, nt*D]
        o = o.reshape(P, nt, D).transpose(1, 0, 2)  # [nt, P, D]
        nid = node_of_all[k]
        m = nid >= 0
        out_full[nid[m]] = o[m]
    has_edge = counts > 0
    out_full[has_edge] += bias
    out_full[~has_edge] = bias0
    return out_full


def kernel(**inputs) -> np.ndarray:
    from concourse.bass_utils import run_bass_kernel_spmd

    in_maps, meta = _prepare(inputs)
    nt, s_list = meta[1], meta[2]
    nc = _build_program(nt, s_list)
    nc.finalize()
    res = run_bass_kernel_spmd(nc, in_maps, core_ids=list(range(NCORES)))
    return _gather(res.results, meta)
